# revision 2
# baseline (speedup 1.0000x reference)
"""Trainium2 Bass kernel v4 for nn_DDI: sequential patch recurrence
    y_i = gelu(W @ y_{i-1} + b) + x_i   (patch=3, over 999 chunks)

Scheme (per core, data-parallel over batch across 8 cores):
  - Split-matmul recurrence (keeps the serial chain short): per step
    psum = W@x_{t-1} (off-chain) + W@g_{t-1} (on-chain), gelu on ACT,
    y = g + x on DVE (output only, off-chain).
  - x-side matmuls in bf16 hi/lo split (3 x 1cyc/row vs 4cyc fp32):
    W@x ~= Whi@xhi + Whi@xlo + Wlo@xhi, accumulated in fp32 PSUM with
    the fp32 W@g matmul. Host stages xhi = bf16(x), xlo = bf16(x-xhi).
    Residual ~2^-18 per step vs fp32's 2^-24; measured noise
    amplification ~70x keeps the final error ~3e-3 << 2e-2.
  - The recurrent g-side matmul stays fp32 (bf16 there diverges).
  - y = g + xhi on DVE (xlo dropped: output-only error ~2e-4).
  - Segment-column layout (FDS=25 cols per segment) with unequal
    segment starts: any (S, WARM); overlapping coverage chunks are
    written twice with converged values. Warmup outputs of segments
    >= 1 are never written (trim).
"""

import os as _os

import numpy as np

import concourse.bass as bass
import concourse.bacc as bacc
import concourse.mybir as mybir
from concourse.tile import TileContext
from concourse.bass_utils import run_bass_kernel_spmd

# ---- problem constants ----
B, SEQ, F = 128, 3000, 64
PATCH = 3
NCH = (SEQ - PATCH) // PATCH  # 999
NCORES = 8
BL = B // NCORES  # 16
G = 42            # groups of 3 partitions
PG = 3 * G        # 126
LPS = BL * F      # 1024 lanes per segment
FDS = -(-LPS // G)  # 25 columns per segment

# ---- tunables ----
S = int(_os.environ.get("DDI_S", "12"))
WARM = int(_os.environ.get("DDI_WARM", "39"))
NCOH = int(_os.environ.get("DDI_NCOH", "3"))
XB = int(_os.environ.get("DDI_XB", "6"))
PSB = int(_os.environ.get("DDI_PSB", "2"))
GPB = int(_os.environ.get("DDI_GPB", "4"))
XPB = int(_os.environ.get("DDI_XPB", "3"))
YPB = int(_os.environ.get("DDI_YPB", "3"))
XBF = int(_os.environ.get("DDI_XBF", "1"))    # bf16 hi/lo x-side matmuls
TRIM = int(_os.environ.get("DDI_TRIM", "0"))
OUT_BF16 = int(_os.environ.get("DDI_BF16", "1"))

assert S % NCOH == 0
SEGC = S // NCOH
FD = SEGC * FDS          # free dim per cohort tile
TR = WARM + -(-(NCH - WARM) // S)
NB = -(-TR // XB)
T = NB * XB
_D = -(-(NCH - TR) // (S - 1)) if S > 1 else 0
STARTS = [min(s * _D, NCH - TR) for s in range(S)]
WB = WARM // XB if TRIM else 0   # fully-warmup batches (trimmed writes)

DT = mybir.dt.float32
DTX = mybir.dt.bfloat16 if XBF else mybir.dt.float32
DTY = mybir.dt.bfloat16 if OUT_BF16 else mybir.dt.float32
XW = 2 if XBF else 1     # hi/lo blocks in the x staging


def _build_nc():
    nc = bacc.Bacc("TRN2", target_bir_lowering=False, debug=False)

    CW = PG + 1 + NCOH * FD
    cst = nc.dram_tensor("cst", [PG, CW], DT, kind="ExternalInput")
    # [Whi | Wlo] kron blocks for the bf16 x-side matmuls
    wbf = nc.dram_tensor("wbf", [PG, 2 * PG], mybir.dt.bfloat16,
                         kind="ExternalInput")
    xs = nc.dram_tensor("xs", [NB, NCOH, PG, XW * XB * FD], DTX,
                        kind="ExternalInput")
    ys = nc.dram_tensor("ys", [NB, NCOH, PG, XB * FD], DTY,
                        kind="ExternalOutput")

    with TileContext(nc) as tc:
        with (
            tc.tile_pool(name="consts", bufs=1) as consts,
            tc.tile_pool(name="xp", bufs=XPB) as xp,
            tc.tile_pool(name="gp", bufs=GPB) as gp,
            tc.tile_pool(name="yp", bufs=YPB) as yp,
            tc.tile_pool(name="ybp", bufs=2) as ybp,
            tc.tile_pool(name="ps", bufs=PSB, space="PSUM") as ps,
            tc.tile_pool(name="wps", bufs=1, space="PSUM") as wps,
        ):
            # Startup: ACT gelu table load + PE p-state ramp during the
            # initial DMA wait.
            warm = consts.tile([PG, 128], DT)
            nc.vector.memset(warm[:], 0.0)
            wpsum = wps.tile([PG, 32], DT, tag="warm")
            for _ in range(20):
                nc.tensor.matmul(wpsum[:], warm[:, 0:PG], warm[:, 0:32],
                                 start=True, stop=True)
            wout = consts.tile([PG, 1], DT)
            nc.scalar.activation(wout[:], warm[:, 0:1],
                                 mybir.ActivationFunctionType.Gelu)

            def emit_out(j, c):
                if TRIM and j < WB and c > 0:
                    return  # warmup outputs of segments >= 1 discarded
                if OUT_BF16:
                    src = ybp.tile([PG, XB * FD], DTY, tag=f"yb{c}")
                    nc.gpsimd.tensor_copy(src[:], y_tiles[c][j][:])
                    src = src[:]
                else:
                    src = y_tiles[c][j][:]
                if TRIM and j < WB:
                    # only segment 0 (first FDS cols per step) is real
                    for ii in range(XB):
                        nc.sync.dma_start(
                            ys[j, 0][:, ii * FD:ii * FD + FDS],
                            src[:, ii * FD:ii * FD + FDS])
                else:
                    nc.sync.dma_start(ys[j, c], src)

            ct = consts.tile([PG, CW], DT)
            nc.sync.dma_start(ct[:], cst[:])
            wT_t = ct[:, 0:PG]
            b_t = ct[:, PG:PG + 1]
            wb = consts.tile([PG, 2 * PG], mybir.dt.bfloat16)
            nc.sync.dma_start(wb[:], wbf[:])
            whi = wb[:, 0:PG]
            wlo = wb[:, PG:2 * PG]

            x_tiles = [[] for _ in range(NCOH)]
            y_tiles = [[] for _ in range(NCOH)]
            psum_cur = [None] * NCOH
            g_prev = [ct[:, PG + 1 + c * FD: PG + 1 + (c + 1) * FD]
                      for c in range(NCOH)]
            for t in range(T):
                j, i = divmod(t, XB)
                if i == 0:
                    for c in range(NCOH):
                        xt = xp.tile([PG, XW * XB * FD], DTX, tag=f"x{c}")
                        nc.sync.dma_start(xt[:], xs[j, c])
                        x_tiles[c].append(xt)
                        yt = yp.tile([PG, XB * FD], DT, tag=f"y{c}")
                        y_tiles[c].append(yt)
                if t >= TR:
                    if i == XB - 1:
                        for c in range(NCOH):
                            emit_out(j, c)
                    continue

                for c in range(NCOH):
                    xt = x_tiles[c][j]
                    xhi_t = xt[:, i * FD:(i + 1) * FD]

                    # close out this step's psum with the on-chain g-matmul
                    if t == 0:
                        psum = ps.tile([PG, FD], DT, tag=f"z{c}")
                        nc.tensor.matmul(psum[:], wT_t, g_prev[c],
                                         start=True, stop=True)
                    else:
                        psum = psum_cur[c]
                        nc.tensor.matmul(psum[:], wT_t, g_prev[c],
                                         start=False, stop=True)

                    # immediately queue next step's off-chain x-side
                    # matmuls so PE's 4-deep wait queue always has
                    # satisfied work behind each waiting g-matmul
                    if t + 1 < TR:
                        pn = ps.tile([PG, FD], DT, tag=f"z{c}")
                        if XBF:
                            xlo_t = xt[:, (XB + i) * FD:(XB + i + 1) * FD]
                            nc.tensor.matmul(pn[:], whi, xhi_t,
                                             start=True, stop=False)
                            nc.tensor.matmul(pn[:], whi, xlo_t,
                                             start=False, stop=False)
                            nc.tensor.matmul(pn[:], wlo, xhi_t,
                                             start=False, stop=False)
                        else:
                            nc.tensor.matmul(pn[:], wT_t, xhi_t,
                                             start=True, stop=False)
                        psum_cur[c] = pn

                    g_t = gp.tile([PG, FD], DT, tag=f"g{c}")
                    nc.scalar.activation(g_t[:], psum[:],
                                         mybir.ActivationFunctionType.Gelu,
                                         bias=b_t)

                    y_slice = y_tiles[c][j][:, i * FD:(i + 1) * FD]
                    nc.vector.tensor_add(y_slice, g_t[:], xhi_t)

                    if i == XB - 1:
                        emit_out(j, c)

                    g_prev[c] = g_t[:]

    nc.compile()
    return nc


_NC_CACHE = None


def _get_nc():
    global _NC_CACHE
    if _NC_CACHE is None:
        _NC_CACHE = _build_nc()
    return _NC_CACHE


def _bf16_f32(a):
    """Round float32 -> bf16 values (kept in float32)."""
    u = np.ascontiguousarray(a, dtype=np.float32).view(np.uint32)
    r = (u + 0x7FFF + ((u >> 16) & 1)) & 0xFFFF0000
    return r.astype(np.uint32).view(np.float32)


def _np_bf16():
    import ml_dtypes
    return ml_dtypes.bfloat16


def _lanes_to_part(flat):
    """flat [..., 1024, 3] -> [..., 126, 25]."""
    lead = flat.shape[:-2]
    out = np.zeros(lead + (G * FDS, PATCH), dtype=flat.dtype)
    out[..., :LPS, :] = flat
    out = out.reshape(lead + (G, FDS, PATCH))
    out = np.swapaxes(out, -1, -2)
    return out.reshape(lead + (PG, FDS))


def _part_to_lanes(tiles):
    """[..., 126, 25] -> [..., 1024, 3]."""
    lead = tiles.shape[:-2]
    arr = tiles.reshape(lead + (G, PATCH, FDS))
    arr = np.swapaxes(arr, -1, -2)
    return arr.reshape(lead + (G * FDS, PATCH))[..., :LPS, :]


def _pack_steps(xt):
    """[T?, NCOH, PG, FD] -> [NB, NCOH, PG, XB*FD] (pad steps to T)."""
    full = np.zeros((T,) + xt.shape[1:], dtype=xt.dtype)
    full[:xt.shape[0]] = xt
    return np.ascontiguousarray(
        full.reshape(NB, XB, NCOH, PG, FD).transpose(0, 2, 3, 1, 4).reshape(
            NB, NCOH, PG, XB * FD))


def _stage_core(xc, W, bvec):
    chunks = xc[:, PATCH:, :].reshape(BL, NCH, PATCH, F)
    cidx = np.asarray(STARTS)[:, None] + np.arange(TR)[None, :]
    arr = chunks[:, cidx, :, :]            # [b, s, t, r, f]
    arr = arr.transpose(2, 1, 0, 4, 3)     # [t, s, b, f, r]
    flat = arr.reshape(TR, S, LPS, PATCH).astype(np.float32)

    def to_tiles(a):
        xt = _lanes_to_part(a)             # [TR, S, 126, 25]
        xt = xt.reshape(TR, NCOH, SEGC, PG, FDS).transpose(0, 1, 3, 2, 4)
        return xt.reshape(TR, NCOH, PG, FD)

    if XBF:
        bf = _np_bf16()
        hi = _bf16_f32(flat)
        lo = (flat - hi).astype(np.float32)
        hi_t = _pack_steps(to_tiles(hi))   # [NB, NCOH, PG, XB*FD]
        lo_t = _pack_steps(to_tiles(lo))
        xsarr = np.concatenate([hi_t, lo_t], axis=3).astype(bf)
        # layout must be [hi-block | lo-block] per batch
    else:
        xsarr = _pack_steps(to_tiles(flat)).astype(np.float32)

    # g0: segment 0 starts from the head; others from zero. The t=0
    # matmul consumes g_prev only (no x side), so stage y0 as "g0".
    y0 = np.zeros((NCOH, PG, FD), dtype=np.float32)
    headflat = xc[:, :PATCH, :].transpose(0, 2, 1).reshape(LPS, PATCH)
    y0[0, :, :FDS] = _lanes_to_part(headflat[None].astype(np.float32))[0]

    wT = np.kron(np.eye(G, dtype=np.float32), W.T.astype(np.float32))
    bcol = np.tile(bvec.astype(np.float32), G)[:, None]
    cst = np.ascontiguousarray(
        np.concatenate([wT, bcol] + [y0[c] for c in range(NCOH)], axis=1),
        dtype=np.float32)

    whi = _bf16_f32(W.astype(np.float32))
    wlo = (W.astype(np.float32) - whi).astype(np.float32)
    wbf = np.concatenate(
        [np.kron(np.eye(G, dtype=np.float32), whi.T),
         np.kron(np.eye(G, dtype=np.float32), wlo.T)],
        axis=1).astype(_np_bf16() if XBF else np.float32)
    if not XBF:
        wbf = wbf.astype(_np_bf16())  # unused but dtype must match
    return {"cst": cst, "xs": xsarr, "wbf": wbf}


def _unstage_core(ys):
    yt = np.asarray(ys, dtype=np.float32)
    yt = yt.reshape(NB, NCOH, PG, XB, FD).transpose(0, 3, 1, 2, 4)
    yt = yt.reshape(T, NCOH, PG, FD)[:TR]
    yt = yt.reshape(TR, NCOH, PG, SEGC, FDS).transpose(0, 1, 3, 2, 4)
    yt = yt.reshape(TR, S, PG, FDS)
    flat = _part_to_lanes(yt)              # [TR, S, 1024, 3]
    arr = flat.reshape(TR, S, BL, F, PATCH)
    out = np.empty((BL, NCH, PATCH, F), dtype=np.float32)
    for s in range(S):
        t0 = 0 if s == 0 else WARM
        out[:, STARTS[s] + t0: STARTS[s] + TR] = arr[t0:TR, s].transpose(
            1, 0, 3, 2)
    return out.reshape(BL, NCH * PATCH, F)


def kernel(x, agg_w, agg_b, _trace=False):
    x = np.asarray(x, dtype=np.float32)
    W = np.asarray(agg_w, dtype=np.float32)
    bvec = np.asarray(agg_b, dtype=np.float32)

    nc = _get_nc()
    in_maps = [_stage_core(x[c * BL:(c + 1) * BL], W, bvec)
               for c in range(NCORES)]
    res = run_bass_kernel_spmd(nc, in_maps, list(range(NCORES)),
                               trace=_trace)

    out = np.empty((B, SEQ, F), dtype=np.float32)
    out[:, :PATCH, :] = x[:, :PATCH, :]
    for c in range(NCORES):
        out[c * BL:(c + 1) * BL, PATCH:, :] = _unstage_core(
            np.asarray(res.results[c]["ys"]))
    if _trace:
        return out, res
    return out
